# revision 1
# baseline (speedup 1.0000x reference)
"""Trainium2 Bass kernel for nn_CHESHIRE (hypergraph GNN message passing).

Strategy (hyperedge-parallel across the 8 cores):
  * The clique-edge structure is a disjoint union of 8-node cliques (one per
    hyperedge), so the normalized Laplacian has the closed form
    lap(v) = (v - group_sum(v)) / 7 and the K=3 Chebyshev conv collapses to
    out = x_gn @ Wx + gsum(x_gn) @ Wg with host-folded weight combos.
  * GraphNorm is a per-hyperedge affine x_gn = x*A_e + B_e folded into the
    same matmuls; only per-hyperedge [EMB] stats are computed on device.
  * Node encodings (and their squares) are computed once per core and stored
    to DRAM as an fp16 [node, x||x^2] table; incidence rows are fetched with
    per-partition indirect DMAs (128 rows each), member-plane-major so all
    per-hyperedge reductions become plane-wise ops: PE identity-matmul
    accumulation for sums, pairwise-max/min trees for the poolings.
"""

import sys

sys.path.insert(0, "/opt/trn_rl_repo")

import numpy as np

import concourse.bacc as bacc
import concourse.bass as bass
import concourse.mybir as mybir
from concourse import tile
from concourse.bass_utils import run_bass_kernel_spmd

F16 = mybir.dt.float16
F32 = mybir.dt.float32
I32 = mybir.dt.int32
AF = mybir.ActivationFunctionType
OP = mybir.AluOpType

# Problem constants (hardcoded per contract).
N, F, EMB, CONV = 2000, 256, 128, 128
E, S = 20000, 8
NCORES = 8
ECORE = E // NCORES          # 2500
EPAD = 2560                  # padded per-core edge count
NBLK = 5
L = EPAD // NBLK             # 512 edges per block
COLS = S * L                 # 4096 gathered columns per block
NG = NBLK * S * (L // 128)   # 160 gather instructions per core
# tapered blocks: long chains amortize early, short chain at the tail
_SIZES = [512, 512, 512, 512, 256, 128, 128]
BLOCKS = []
_o = 0
for _l in _SIZES:
    BLOCKS.append((_o, _l))
    _o += _l
assert _o == EPAD
NPAD = 2048                  # padded node count
EPS = 1e-5

_CACHE = {}


def _build_program():
    nc = bacc.Bacc(None, target_bir_lowering=False, debug=False)

    featT_d = nc.dram_tensor("featT", [F, NPAD], F16, kind="ExternalInput")
    wenc_d = nc.dram_tensor("wenc", [F, EMB], F16, kind="ExternalInput")
    benc_d = nc.dram_tensor("benc", [1, EMB], F16, kind="ExternalInput")
    wx_d = nc.dram_tensor("wx", [EMB, CONV], F16, kind="ExternalInput")
    wu_d = nc.dram_tensor("wu", [EMB, CONV], F16, kind="ExternalInput")
    ww_d = nc.dram_tensor("ww", [EMB, CONV], F16, kind="ExternalInput")
    wo_d = nc.dram_tensor("wo", [CONV, 2], F16, kind="ExternalInput")
    eyef_d = nc.dram_tensor("eyef", [128, 128], F16, kind="ExternalInput")
    eye32_d = nc.dram_tensor("eye32", [128, 128], F32, kind="ExternalInput")
    vecs_d = nc.dram_tensor("vecs", [128, 8], F32, kind="ExternalInput")
    idx_d = nc.dram_tensor("idx32", [128, NG], I32, kind="ExternalInput")
    yout_d = nc.dram_tensor("yout", [EPAD], F32, kind="ExternalOutput")

    xcat_d = nc.dram_tensor("xcat_scratch", [NPAD, 2 * EMB], F16)

    with tile.TileContext(nc) as tc:
        with (
            tc.tile_pool(name="weights", bufs=1) as wpool,
            tc.tile_pool(name="smalls", bufs=1) as spool,
            tc.tile_pool(name="gath", bufs=1) as gpool,
            tc.tile_pool(name="big", bufs=2) as bigp,
            tc.tile_pool(name="psA", bufs=1, space="PSUM") as psA,
            tc.tile_pool(name="psB", bufs=1, space="PSUM") as psB,
        ):
            # ---- load weights / tables ----
            featT0 = wpool.tile([128, NPAD], F16, tag="featT0")
            featT1 = wpool.tile([128, NPAD], F16, tag="featT1")
            nc.sync.dma_start(featT0[:], featT_d[0:128, :])
            nc.sync.dma_start(featT1[:], featT_d[128:256, :])
            wenc0 = wpool.tile([128, EMB], F16, tag="wenc0")
            wenc1 = wpool.tile([128, EMB], F16, tag="wenc1")
            nc.sync.dma_start(wenc0[:], wenc_d[0:128, :])
            nc.sync.dma_start(wenc1[:], wenc_d[128:256, :])
            benc = wpool.tile([1, EMB], F16, tag="benc")
            nc.sync.dma_start(benc[:], benc_d[:])
            wx = wpool.tile([EMB, CONV], F16, tag="wx")
            nc.sync.dma_start(wx[:], wx_d[:])
            wu = wpool.tile([EMB, CONV], F16, tag="wu")
            nc.sync.dma_start(wu[:], wu_d[:])
            ww = wpool.tile([EMB, CONV], F16, tag="ww")
            nc.sync.dma_start(ww[:], ww_d[:])
            wo = wpool.tile([CONV, 2], F16, tag="wo")
            nc.sync.dma_start(wo[:], wo_d[:])
            eyef = wpool.tile([128, 128], F16, tag="eyef")
            nc.sync.dma_start(eyef[:], eyef_d[:])
            eye32 = wpool.tile([128, 128], F32, tag="eye32")
            nc.sync.dma_start(eye32[:], eye32_d[:])
            vecs = wpool.tile([128, 8], F32, tag="vecs")
            nc.sync.dma_start(vecs[:], vecs_d[:])
            idx = wpool.tile([128, NG], I32, tag="idx")
            nc.sync.dma_start(idx[:], idx_d[:])
            ones = wpool.tile([1, 128], F16, tag="ones")
            nc.vector.memset(ones[:], 1.0)

            c2v = vecs[:, 0:1]     # (2s - s^2)/8
            wgv = vecs[:, 1:2]     # gn_weight
            s8v = vecs[:, 2:3]     # gn_mean_scale/8
            cconv = vecs[:, 3:4]   # c_const (+cheb_b) per CONV feature
            boutv = vecs[0:1, 4:5]  # b_out scalar

            # ---- encoder: x_enc = clip(feat @ W_enc + b_enc) -> fp16 tables
            xenc = wpool.tile([128, NPAD], F16, tag="xenc")
            xsq = wpool.tile([128, NPAD], F16, tag="xsq")
            for g in range(4):
                ep = psB.tile([128, 512], F32, tag="vp", name="ep", bufs=2)
                for t4 in range(4):
                    t = 4 * g + t4
                    sl = bass.ts(t, 128)
                    out = ep[:, bass.ts(t4, 128)]
                    nc.tensor.matmul(out, featT0[:, sl], wenc0[:],
                                     start=True, stop=False)
                    nc.tensor.matmul(out, featT1[:, sl], wenc1[:],
                                     start=False, stop=False)
                    nc.tensor.matmul(out, ones[:], benc[:],
                                     start=False, stop=True)
                nc.vector.tensor_scalar(xenc[:, bass.ts(g, 512)], ep[:],
                                        1.0, -1.0, op0=OP.min, op1=OP.max)
                nc.scalar.activation(xsq[:, bass.ts(g, 512)],
                                     xenc[:, bass.ts(g, 512)], AF.Square)
                # store this 512-node slice of the [x || x^2] table
                r0 = g * 512
                nc.sync.dma_start(
                    xcat_d[r0:r0 + 512, 0:EMB].rearrange(
                        "(t p) e -> p t e", p=128),
                    xenc[:, bass.ts(g, 512)].rearrange(
                        "p (t e) -> p t e", e=128),
                )
                nc.sync.dma_start(
                    xcat_d[r0:r0 + 512, EMB:2 * EMB].rearrange(
                        "(t p) e -> p t e", p=128),
                    xsq[:, bass.ts(g, 512)].rearrange(
                        "p (t e) -> p t e", e=128),
                )

            logit = wpool.tile([1, EPAD], F32, tag="logit")

            tcol = 0
            for b, (e0, Lb) in enumerate(BLOCKS):
                # ---- gather 4096 incidence rows (row-major, [x || x^2]) ----
                xg = []  # xg[j]: [128 edges, 4 quarters, 256] fp16
                for j in range(S):
                    g_j = gpool.tile([128, Lb // 128, 2 * EMB], F16, tag=f"xg{b}_{j}",
                                     name=f"xg{b}_{j}")
                    for q in range(Lb // 128):
                        t = tcol + j * (Lb // 128) + q
                        nc.gpsimd.indirect_dma_start(
                            out=g_j[:, q, :], out_offset=None, in_=xcat_d[:],
                            in_offset=bass.IndirectOffsetOnAxis(
                                ap=idx[:, t:t + 1], axis=0))
                    xg.append(g_j)

                # ---- transpose x to feature-major in the gather shadow
                xT = bigp.tile([128, S * Lb], F16, tag="xT", bufs=1)
                for j in range(S):
                    xtp = psB.tile([128, Lb], F16, tag="xtp", bufs=2)
                    for q in range(Lb // 128):
                        nc.tensor.transpose(xtp[:, bass.ts(q, 128)],
                                            xg[j][:, q, 0:EMB], eyef[:])
                    nc.scalar.activation(xT[:, bass.ts(j, Lb)], xtp[:],
                                         AF.Identity)

                # ---- per-edge sums over the 8 member planes (PE, row-major)
                g8rm = spool.tile([128, Lb], F32, tag="g8rm")
                q8rm = spool.tile([128, Lb], F32, tag="q8rm")
                gp = psA.tile([128, Lb], F32, tag="gp")
                qp = psA.tile([128, Lb], F32, tag="qp")
                for j in range(S):
                    nc.tensor.matmul(gp[:], eyef[:], xg[j][:, 0:Lb // 128, 0:EMB],
                                     start=(j == 0), stop=(j == S - 1))
                for j in range(S):
                    nc.tensor.matmul(qp[:], eyef[:], xg[j][:, 0:Lb // 128, EMB:2 * EMB],
                                     start=(j == 0), stop=(j == S - 1))
                nc.scalar.activation(g8rm[:], gp[:], AF.Identity)
                nc.scalar.activation(q8rm[:], qp[:], AF.Identity)

                # transpose per-edge stats to feature-major [EMB, 512]
                g8tp = psA.tile([128, Lb], F32, tag="gp", name="g8tp")
                q8tp = psA.tile([128, Lb], F32, tag="qp", name="q8tp")
                for q in range(Lb // 128):
                    nc.tensor.transpose(g8tp[:, bass.ts(q, 128)],
                                        g8rm[:, bass.ts(q, 128)], eye32[:])
                    nc.tensor.transpose(q8tp[:, bass.ts(q, 128)],
                                        q8rm[:, bass.ts(q, 128)], eye32[:])
                g8s = spool.tile([128, Lb], F32, tag="g8s")
                nc.scalar.activation(g8s[:], g8tp[:], AF.Identity)

                # GraphNorm per-hyperedge affine: A = w / sqrt(var+eps)
                t1 = spool.tile([128, Lb], F32, tag="t1")
                nc.scalar.activation(t1[:], g8s[:], AF.Square)
                vx8 = spool.tile([128, Lb], F32, tag="vx8")
                nc.vector.scalar_tensor_tensor(vx8[:], t1[:], vecs[:, 6:7],
                                               q8tp[:], op0=OP.mult,
                                               op1=OP.add)
                vc = spool.tile([128, Lb], F32, tag="vc")
                nc.vector.tensor_scalar(vc[:], vx8[:], 0.0, 8.0 * EPS,
                                        op0=OP.max, op1=OP.add)
                ex = spool.tile([128, Lb], F32, tag="ex")
                nc.scalar.activation(ex[:], vc[:], AF.Abs_reciprocal_sqrt,
                                     scale=0.125)
                A = spool.tile([128, Lb], F16, tag="A")
                nc.vector.tensor_scalar(A[:], ex[:], wgv, None, op0=OP.mult)
                w8 = spool.tile([128, Lb], F16, tag="w8")
                nc.vector.scalar_tensor_tensor(w8[:], ex[:], wgv, g8s[:],
                                               op0=OP.mult, op1=OP.mult)
                u = spool.tile([128, Lb], F16, tag="u")
                nc.vector.tensor_scalar(u[:], w8[:], s8v, None, op0=OP.mult)

                # per-hyperedge C = u @ Wu + w8 @ Ww
                cp = psB.tile([128, Lb], F32, tag="cpspfp", name="cp")
                nc.tensor.matmul(cp[:], wu[:], u[:], start=True, stop=False)
                nc.tensor.matmul(cp[:], ww[:], w8[:], start=False, stop=True)
                cs = spool.tile([128, Lb], F16, tag="cs")
                nc.scalar.activation(cs[:], cp[:], AF.Identity, bias=cconv)

                # ---- apply A (broadcast over planes), cheb matmul ----
                z = bigp.tile([128, S * Lb], F16, tag="z", bufs=1)
                rhs = bigp.tile([128, S * Lb], F16, tag="rhs", bufs=1)
                nc.vector.tensor_tensor(
                    rhs[:].rearrange("p (j c) -> p j c", j=S),
                    xT[:].rearrange("p (j c) -> p j c", j=S),
                    A[:].unsqueeze(1).broadcast_to([128, S, Lb]),
                    op=OP.mult)
                for j in range(S):
                    vp = psB.tile([128, Lb], F32, tag="vp", bufs=2)
                    nc.tensor.matmul(vp[:], wx[:], rhs[:, bass.ts(j, Lb)],
                                     start=True, stop=True)
                    # egress + per-edge C (and c_const, folded into cs) add
                    nc.vector.tensor_tensor(z[:, bass.ts(j, Lb)], vp[:],
                                            cs[:], op=OP.add)

                zc = bigp.tile([128, S * Lb], F16, tag="zc", bufs=1)
                nc.vector.tensor_scalar(zc[:], z[:], 1.0, -1.0,
                                        op0=OP.min, op1=OP.max)

                # ---- poolings over the 8 planes ----
                pl = [zc[:, bass.ts(j, Lb)] for j in range(S)]
                mx = [spool.tile([128, Lb], F16, tag=f"mx{k}", name=f"mx{k}")
                      for k in range(4)]
                mn = [spool.tile([128, Lb], F16, tag=f"mn{k}", name=f"mn{k}")
                      for k in range(4)]
                for k in range(4):
                    nc.vector.tensor_tensor(mx[k][:], pl[2 * k], pl[2 * k + 1],
                                            op=OP.max)
                    nc.vector.tensor_tensor(mn[k][:], pl[2 * k], pl[2 * k + 1],
                                            op=OP.min)
                mx2a = spool.tile([128, Lb], F16, tag="mx2a")
                mx2b = spool.tile([128, Lb], F16, tag="mx2b")
                mn2a = spool.tile([128, Lb], F16, tag="mn2a")
                mn2b = spool.tile([128, Lb], F16, tag="mn2b")
                nc.vector.tensor_tensor(mx2a[:], mx[0][:], mx[1][:], op=OP.max)
                nc.vector.tensor_tensor(mx2b[:], mx[2][:], mx[3][:], op=OP.max)
                nc.vector.tensor_tensor(mn2a[:], mn[0][:], mn[1][:], op=OP.min)
                nc.vector.tensor_tensor(mn2b[:], mn[2][:], mn[3][:], op=OP.min)
                zmax = spool.tile([128, Lb], F16, tag="zmax")
                zmin = spool.tile([128, Lb], F16, tag="zmin")
                nc.vector.tensor_tensor(zmax[:], mx2a[:], mx2b[:], op=OP.max)
                nc.vector.tensor_tensor(zmin[:], mn2a[:], mn2b[:], op=OP.min)
                rng = spool.tile([128, Lb], F16, tag="rng")
                nc.vector.tensor_tensor(rng[:], zmax[:], zmin[:],
                                        op=OP.subtract)

                sq2 = bigp.tile([128, S * Lb], F16, tag="sq2", bufs=1)
                nc.scalar.activation(sq2[:], zc[:], AF.Square)
                sp = psB.tile([128, Lb], F32, tag="cpspfp", name="sp")
                for j in range(S):
                    nc.tensor.matmul(sp[:], eyef[:], sq2[:, bass.ts(j, Lb)],
                                     start=(j == 0), stop=(j == S - 1))
                # ynorm = sqrt(ssq/8) = (ssq/8) * rsqrt(ssq/8)
                r2 = spool.tile([128, Lb], F32, tag="r2")
                nc.scalar.activation(r2[:], sp[:], AF.Abs_reciprocal_sqrt,
                                     scale=0.125, bias=vecs[:, 5:6])
                ynorm = spool.tile([128, Lb], F16, tag="ynorm")
                nc.vector.scalar_tensor_tensor(ynorm[:], sp[:], 0.125, r2[:],
                                               op0=OP.mult, op1=OP.mult)

                fp = psB.tile([1, Lb], F32, tag="cpspfp", name="fp")
                nc.tensor.matmul(fp[:], wo[:, 0:1], rng[:],
                                 start=True, stop=False)
                nc.tensor.matmul(fp[:], wo[:, 1:2], ynorm[:],
                                 start=False, stop=True)
                nc.scalar.activation(logit[0:1, e0:e0 + Lb], fp[:],
                                     AF.Identity)

                tcol += S * (Lb // 128)

            ysb = wpool.tile([1, EPAD], F32, tag="ysb")
            nc.scalar.activation(ysb[:], logit[:], AF.Sigmoid, bias=boutv)
            nc.sync.dma_start(yout_d[:].rearrange("(p c) -> p c", p=1), ysb[:])

    nc.compile()
    return nc


def _get_program():
    if "nc" not in _CACHE:
        _CACHE["nc"] = _build_program()
    return _CACHE["nc"]


def _host_prep(inputs):
    """Fold weights and stage per-core input maps."""
    f = lambda k: np.asarray(inputs[k], np.float32)
    feature = f("feature")
    W_enc, b_enc = f("W_enc"), f("b_enc")
    gw, gb, gs = f("gn_weight"), f("gn_bias"), f("gn_mean_scale")
    cheb_W = np.asarray(inputs["cheb_W"], np.float64)
    cheb_b = np.asarray(inputs["cheb_b"], np.float64)
    W_out, b_out = f("W_out"), f("b_out")
    hn = np.asarray(inputs["hyperedge_nodes"]).astype(np.int64)

    d = float(S - 1)
    W0, W1, W2 = cheb_W[0], cheb_W[1], cheb_W[2]
    Wx64 = W0 + W1 / d + W2 * ((2.0 - d * d) / (d * d))
    Wg64 = -W1 / d + W2 * (2.0 * (d - 1.0) / (d * d))
    c_const = (gb.astype(np.float64) @ (Wx64 + S * Wg64) + cheb_b)

    featT = np.zeros((F, NPAD), np.float16)
    featT[:, :N] = feature.T.astype(np.float16)
    wenc = W_enc.astype(np.float16)
    benc = b_enc.reshape(1, EMB).astype(np.float16)
    wx16 = Wx64.astype(np.float16)
    wu16 = (-(Wx64 + S * Wg64)).astype(np.float16)
    ww16 = Wg64.astype(np.float16)
    wo16 = np.stack([W_out[:CONV, 0], W_out[CONV:, 0]], axis=1).astype(np.float16)
    eyef = np.eye(128, dtype=np.float16)
    eye32 = np.eye(128, dtype=np.float32)
    vecs = np.zeros((128, 8), np.float32)
    vecs[:, 0] = (2.0 * gs - gs * gs) / 8.0
    vecs[:, 1] = gw
    vecs[:, 2] = gs / 8.0
    vecs[:, 3] = c_const.astype(np.float32)
    vecs[0, 4] = b_out[0]
    vecs[:, 5] = 1e-30
    vecs[:, 6] = -(2.0 * gs - gs * gs) / 8.0

    shared = dict(featT=featT, wenc=wenc, benc=benc, wx=wx16, wu=wu16,
                  ww=ww16, wo=wo16, eyef=eyef, eye32=eye32, vecs=vecs)

    in_maps = []
    for c in range(NCORES):
        base = c * ECORE
        hcol = np.zeros((EPAD, S), np.int32)
        hcol[:ECORE] = hn[base:base + ECORE].astype(np.int32)
        # gather t = b*32 + j*4 + q covers edges [b*512+q*128, +128), member j
        idx = np.zeros((128, NG), np.int32)
        t = 0
        for e0, lb in BLOCKS:
            for j in range(S):
                for q in range(lb // 128):
                    idx[:, t] = hcol[e0 + q * 128:e0 + q * 128 + 128, j]
                    t += 1
        in_maps.append(dict(shared, idx32=idx))
    return in_maps


def _install_trace_hook():
    """Best-effort NTFF profiling under axon (test/benchmark only)."""
    import types
    ah = sys.modules.get("antenv.axon_hooks")
    if ah is None:
        ah = types.ModuleType("antenv.axon_hooks")
        ah._HOOK = None
        ah.set_axon_ntff_profile_hook = lambda h: setattr(ah, "_HOOK", h)
        ah.get_axon_ntff_profile_hook = lambda: ah._HOOK
        sys.modules["antenv.axon_hooks"] = ah
        import antenv
        antenv.axon_hooks = ah
    if ah.get_axon_ntff_profile_hook() is None:
        from trn_agent_boot.trn_boot import _ntff_profile_via_ctypes
        hook = _ntff_profile_via_ctypes("/opt/axon/libaxon_pjrt.so")
        if hook is not None:
            ah.set_axon_ntff_profile_hook(hook)
    import concourse.bass_utils as bu
    bu.upload_artifacts = lambda tmpdir: f"local:{tmpdir}"


def _run(in_maps, trace=False):
    nc = _get_program()
    if trace:
        _install_trace_hook()
    return run_bass_kernel_spmd(nc, in_maps, list(range(NCORES)), trace=trace)


def kernel(**inputs) -> np.ndarray:
    in_maps = _host_prep(inputs)
    res = _run(in_maps)
    out = np.concatenate([res.results[c]["yout"][:ECORE] for c in range(NCORES)])
    return out.reshape(E, 1).astype(np.float32)


def kernel_traced(**inputs):
    """Like kernel() but returns (output, exec_time_ns) using a profiled run."""
    in_maps = _host_prep(inputs)
    res = _run(in_maps, trace=True)
    out = np.concatenate([res.results[c]["yout"][:ECORE] for c in range(NCORES)])
    return out.reshape(E, 1).astype(np.float32), res.exec_time_ns



# revision 22
# speedup vs baseline: 1.1885x; 1.1885x over previous
"""Trainium2 Bass kernel for nn_CHESHIRE (hypergraph GNN message passing).

Strategy (hyperedge-parallel across the 8 cores):
  * Clique Laplacian has the closed form lap(v) = (v - gsum(v))/7, so the
    K=3 Chebyshev conv collapses to z_j = (A.x_j) @ Wx + w8 @ WC + c_const
    with host-folded weight combos (A, w8 per-hyperedge GraphNorm terms).
  * The encoder output [x || x^2] lives in SBUF as a node-major token table;
    incidence rows are fetched feature-major with ONE transposing SBUF-source
    dma_gather per 512-edge block (4096 descriptors amortize the ~1us SWDGE
    fixed cost, and the transpose removes all PE transpose traffic).
  * Per-edge sums (g8/q8) use a single accumulating identity matmul with a
    stride-0 revisit output AP; per-edge C is accumulated straight into the
    cheb PSUM the same way, so PSUM egress is a scalar-engine activation with
    a per-partition bias and the DVE never reads PSUM for the z path.
  * Max/min poolings are fp16 tensor_tensor trees on DVE; ssq pooling is
    another revisit matmul.
"""

import sys

sys.path.insert(0, "/opt/trn_rl_repo")

import numpy as np

import concourse.bacc as bacc
import concourse.bass as bass
import concourse.mybir as mybir
from concourse import tile
from concourse.bass_utils import run_bass_kernel_spmd

F16 = mybir.dt.float16
F32 = mybir.dt.float32
I16 = mybir.dt.int16
AF = mybir.ActivationFunctionType
OP = mybir.AluOpType

# Problem constants (hardcoded per contract).
N, F, EMB, CONV = 2000, 256, 128, 128
E, S = 20000, 8
NCORES = 8
ECORE = E // NCORES          # 2500
EPAD = 2560                  # padded per-core edge count
NBLK = 5
L = EPAD // NBLK             # 512 edges per block
NIDX = S * L                 # 4096 gathered rows per block
NPAD = 2048                  # padded node count (16 ranks of 128)
RANKS = NPAD // 128
EPS = 1e-5

_CACHE = {}


def _build_program():
    nc = bacc.Bacc(None, target_bir_lowering=False, debug=False)

    featT_d = nc.dram_tensor("featT", [F, NPAD], F16, kind="ExternalInput")
    wenc_d = nc.dram_tensor("wenc", [F, EMB], F16, kind="ExternalInput")
    benc_d = nc.dram_tensor("benc", [1, EMB], F16, kind="ExternalInput")
    wx_d = nc.dram_tensor("wx", [EMB, CONV], F16, kind="ExternalInput")
    wc_d = nc.dram_tensor("wc", [EMB, CONV], F16, kind="ExternalInput")
    wo_d = nc.dram_tensor("wo", [CONV, 2], F16, kind="ExternalInput")
    eyef_d = nc.dram_tensor("eyef", [128, 128], F16, kind="ExternalInput")
    vecs_d = nc.dram_tensor("vecs", [128, 8], F32, kind="ExternalInput")
    idx_d = nc.dram_tensor("idx16", [128, NBLK * NIDX // 16], I16,
                           kind="ExternalInput")
    yout_d = nc.dram_tensor("yout", [EPAD], F32, kind="ExternalOutput")

    with tile.TileContext(nc) as tc:
        with (
            tc.tile_pool(name="weights", bufs=1) as wpool,
            tc.tile_pool(name="smalls", bufs=2) as spool,
            tc.tile_pool(name="gath", bufs=2) as gpool,
            tc.tile_pool(name="big", bufs=2) as bigp,
            tc.tile_pool(name="psV", bufs=2, space="PSUM") as psV,
            tc.tile_pool(name="psG", bufs=1, space="PSUM") as psG,
        ):
            # ---- load weights / tables ----
            featT0 = wpool.tile([128, NPAD], F16, tag="featT0")
            featT1 = wpool.tile([128, NPAD], F16, tag="featT1")
            nc.sync.dma_start(featT0[:], featT_d[0:128, :])
            nc.sync.dma_start(featT1[:], featT_d[128:256, :])
            wenc0 = wpool.tile([128, EMB], F16, tag="wenc0")
            wenc1 = wpool.tile([128, EMB], F16, tag="wenc1")
            nc.sync.dma_start(wenc0[:], wenc_d[0:128, :])
            nc.sync.dma_start(wenc1[:], wenc_d[128:256, :])
            benc = wpool.tile([1, EMB], F16, tag="benc")
            nc.sync.dma_start(benc[:], benc_d[:])
            wx = wpool.tile([EMB, CONV], F16, tag="wx")
            nc.sync.dma_start(wx[:], wx_d[:])
            wc = wpool.tile([EMB, CONV], F16, tag="wc")
            nc.sync.dma_start(wc[:], wc_d[:])
            wo = wpool.tile([CONV, 2], F16, tag="wo")
            nc.sync.dma_start(wo[:], wo_d[:])
            vecs = wpool.tile([128, 8], F32, tag="vecs")
            nc.sync.dma_start(vecs[:], vecs_d[:])
            idx = wpool.tile([128, NBLK * NIDX // 16], I16, tag="idx")
            nc.sync.dma_start(idx[:], idx_d[:])
            ones = wpool.tile([1, 128], F16, tag="ones")
            nc.vector.memset(ones[:], 1.0)

            c2v = vecs[:, 0:1]      # (2s - s^2)/8 per EMB feature
            wgv = vecs[:, 1:2]      # gn_weight
            cconv = vecs[:, 3:4]    # c_const (+cheb_b) per CONV feature
            boutv = vecs[0:1, 4:5]  # b_out scalar
            tinyv = vecs[:, 5:6]    # 1e-30
            epsv = vecs[:, 6:7]     # EPS

            # ---- encoder -> node-major [x || x^2] token table in SBUF ----
            # table[p, r*256 + e]       = clip(xenc)[node r*128+p, e]
            # table[p, r*256 + 128 + e] = clip(xenc)^2[node r*128+p, e]
            table = wpool.tile([128, RANKS * 2 * EMB], F16, tag="table")
            for g in range(4):
                ep = psG.tile([128, 512], F32, tag="sp", name=f"ep{g}")
                for t4 in range(4):
                    t = 4 * g + t4
                    sl = bass.ts(t, 128)
                    out = ep[:, bass.ts(t4, 128)]
                    nc.tensor.matmul(out, featT0[:, sl], wenc0[:],
                                     start=True, stop=False)
                    nc.tensor.matmul(out, featT1[:, sl], wenc1[:],
                                     start=False, stop=False)
                    nc.tensor.matmul(out, ones[:], benc[:],
                                     start=False, stop=True)
                xv = table[:].rearrange("p (r h e) -> p r h e", r=RANKS, h=2)
                xs = xv[:, 4 * g:4 * g + 4, 0, :]   # [128, 4, 128] x slots
                qs = xv[:, 4 * g:4 * g + 4, 1, :]   # [128, 4, 128] x^2 slots
                nc.vector.tensor_scalar(
                    xs, ep[:].rearrange("p (r e) -> p r e", r=4),
                    1.0, -1.0, op0=OP.min, op1=OP.max)
                nc.scalar.activation(qs, xs, AF.Square)

            for b in range(NBLK):
                # ---- transposed SBUF-source gathers, one per member plane
                # (the SWDGE ring caps a gather at ~1000 descriptors, so a
                # 512-idx gather per plane is the largest aligned unit).
                # xg[p, j, 0, e] = x[feat p, node(edge e, member j)]
                # xg[p, j, 1, e] = x^2[...]
                xg = gpool.tile([128, S, 2, L], F16, tag="xg")
                for j in range(S):
                    nc.gpsimd.dma_gather(
                        out_ap=xg[:, j, :, :],
                        in_ap=table[:],
                        idxs_ap=idx[:, bass.ts(b * S + j, L // 16)],
                        num_idxs=L,
                        num_idxs_reg=L,
                        elem_size=2 * EMB,
                        transpose=True,
                        sbuf_tokens_per_rank=128,
                        sbuf_free_dim_per_rank=2 * EMB * 2,
                        sbuf_free_dim_pad_per_rank=0,
                        sbuf_byte_offset=0,
                    )
                xgx = xg[:, :, 0, :]  # [128, S, L]

                # ---- per-edge sums over the 8 member planes (x and x^2 at
                # once): fused fp16 add-tree, final level in fp32.
                gq1 = bigp.tile([128, 4, 2, L], F16, tag="gq1")
                nc.vector.tensor_tensor(gq1[:], xg[:, 0:4], xg[:, 4:8],
                                        op=OP.add)
                gq2 = spool.tile([128, 2, 2, L], F16, tag="gq2")
                nc.vector.tensor_tensor(gq2[:], gq1[:, 0:2], gq1[:, 2:4],
                                        op=OP.add)
                gqs = spool.tile([128, 2, L], F32, tag="gqs")
                nc.vector.tensor_tensor(gqs[:], gq2[:, 0], gq2[:, 1],
                                        op=OP.add)
                g8s = gqs[:, 0, :]
                q8s = gqs[:, 1, :]

                # GraphNorm per-hyperedge scale A = gn_w / sqrt(var + eps)
                t1 = spool.tile([128, L], F32, tag="t1")
                nc.scalar.activation(t1[:], g8s, AF.Square)
                t2 = spool.tile([128, L], F32, tag="t2")
                nc.vector.tensor_scalar(t2[:], t1[:], c2v, None, op0=OP.mult)
                vx8 = spool.tile([128, L], F32, tag="vx8")
                nc.vector.tensor_tensor(vx8[:], q8s, t2[:], op=OP.subtract)
                ex = spool.tile([128, L], F32, tag="ex")
                nc.scalar.activation(ex[:], vx8[:], AF.Abs_reciprocal_sqrt,
                                     scale=0.125, bias=epsv)
                A16 = spool.tile([128, L], F16, tag="A16")
                nc.vector.tensor_scalar(A16[:], ex[:], wgv, None, op0=OP.mult)
                w8 = spool.tile([128, L], F16, tag="w8")
                nc.vector.tensor_tensor(w8[:], A16[:], g8s, op=OP.mult)

                # ---- rhs = A (.) x, broadcast A over the 8 member planes
                rhs = bigp.tile([128, S, L], F16, tag="rhs")
                nc.vector.tensor_tensor(
                    rhs[:], xgx, A16[:].unsqueeze(1).broadcast_to([128, S, L]),
                    op=OP.mult)

                # ---- cheb + per-edge C in PSUM; egress with c_const bias
                z = bigp.tile([128, S, L], F16, tag="z")
                for w in range(4):
                    vp = psV.tile([128, 2, L], F32, tag="vp", name=f"vp{b}_{w}")
                    nc.tensor.matmul(vp[:, 0, :], wx[:], rhs[:, 2 * w, :],
                                     start=True, stop=False)
                    nc.tensor.matmul(vp[:, 1, :], wx[:], rhs[:, 2 * w + 1, :],
                                     start=True, stop=False)
                    nc.tensor.matmul(vp[:, 0, :], wc[:], w8[:],
                                     start=False, stop=True)
                    nc.tensor.matmul(vp[:, 1, :], wc[:], w8[:],
                                     start=False, stop=True)
                    nc.scalar.activation(z[:, 2 * w:2 * w + 2, :], vp[:],
                                         AF.Identity, bias=cconv)

                # ---- poolings over the 8 planes (fp16 DVE trees) ----
                mx1 = bigp.tile([128, 4, L], F16, tag="mx1")
                mn1 = bigp.tile([128, 4, L], F16, tag="mn1")
                nc.vector.tensor_tensor(mx1[:], z[:, 0:4, :], z[:, 4:8, :],
                                        op=OP.max)
                nc.vector.tensor_tensor(mn1[:], z[:, 0:4, :], z[:, 4:8, :],
                                        op=OP.min)
                mx2 = spool.tile([128, 2, L], F16, tag="mx2")
                mn2 = spool.tile([128, 2, L], F16, tag="mn2")
                nc.vector.tensor_tensor(mx2[:], mx1[:, 0:2, :], mx1[:, 2:4, :],
                                        op=OP.max)
                nc.vector.tensor_tensor(mn2[:], mn1[:, 0:2, :], mn1[:, 2:4, :],
                                        op=OP.min)
                zmax = spool.tile([128, L], F16, tag="zmax")
                zmin = spool.tile([128, L], F16, tag="zmin")
                nc.vector.tensor_tensor(zmax[:], mx2[:, 0, :], mx2[:, 1, :],
                                        op=OP.max)
                nc.vector.tensor_tensor(zmin[:], mn2[:, 0, :], mn2[:, 1, :],
                                        op=OP.min)
                # rng = clip(zmax) - clip(zmin)
                zmaxc = spool.tile([128, L], F16, tag="zmaxc")
                zminc = spool.tile([128, L], F16, tag="zminc")
                nc.vector.tensor_scalar(zmaxc[:], zmax[:], 1.0, -1.0,
                                        op0=OP.min, op1=OP.max)
                nc.vector.tensor_scalar(zminc[:], zmin[:], 1.0, -1.0,
                                        op0=OP.min, op1=OP.max)
                rng = spool.tile([128, L], F16, tag="rng")
                nc.vector.tensor_tensor(rng[:], zmaxc[:], zminc[:],
                                        op=OP.subtract)

                # ---- ynorm = sqrt(mean_j min(z^2, 1)) ----
                sqm = bigp.tile([128, S, L], F16, tag="rhs", name=f"sqm{b}")
                nc.scalar.activation(sqm[:], z[:], AF.Square)
                nc.vector.tensor_scalar(sqm[:], sqm[:], 1.0, None, op0=OP.min)
                s1 = bigp.tile([128, 4, L], F16, tag="mx1", name=f"s1{b}")
                nc.vector.tensor_tensor(s1[:], sqm[:, 0:4, :], sqm[:, 4:8, :],
                                        op=OP.add)
                s2 = spool.tile([128, 2, L], F16, tag="mx2", name=f"s2{b}")
                nc.vector.tensor_tensor(s2[:], s1[:, 0:2, :], s1[:, 2:4, :],
                                        op=OP.add)
                ssq = spool.tile([128, L], F32, tag="ssq")
                nc.vector.tensor_tensor(ssq[:], s2[:, 0, :], s2[:, 1, :],
                                        op=OP.add)
                r2 = spool.tile([128, L], F32, tag="r2")
                nc.scalar.activation(r2[:], ssq[:], AF.Abs_reciprocal_sqrt,
                                     scale=0.125, bias=tinyv)
                ynorm = spool.tile([128, L], F16, tag="ynorm")
                nc.vector.scalar_tensor_tensor(ynorm[:], ssq[:], 0.125, r2[:],
                                               op0=OP.mult, op1=OP.mult)

                fp = psG.tile([1, L], F32, tag="fp", name=f"fp{b}")
                nc.tensor.matmul(fp[:], wo[:, 0:1], rng[:],
                                 start=True, stop=False)
                nc.tensor.matmul(fp[:], wo[:, 1:2], ynorm[:],
                                 start=False, stop=True)
                yb = spool.tile([1, L], F32, tag="yb", name=f"yb{b}")
                nc.scalar.activation(yb[:], fp[:], AF.Sigmoid, bias=boutv)
                nc.sync.dma_start(
                    yout_d[b * L:(b + 1) * L].rearrange("(p c) -> p c", p=1),
                    yb[:])

    nc.compile()
    return nc


def _get_program():
    if "nc" not in _CACHE:
        _CACHE["nc"] = _build_program()
    return _CACHE["nc"]


def _host_prep(inputs):
    """Fold weights and stage per-core input maps."""
    f = lambda k: np.asarray(inputs[k], np.float32)
    feature = f("feature")
    W_enc, b_enc = f("W_enc"), f("b_enc")
    gw, gb, gs = f("gn_weight"), f("gn_bias"), f("gn_mean_scale")
    cheb_W = np.asarray(inputs["cheb_W"], np.float64)
    cheb_b = np.asarray(inputs["cheb_b"], np.float64)
    W_out, b_out = f("W_out"), f("b_out")
    hn = np.asarray(inputs["hyperedge_nodes"]).astype(np.int64)

    d = float(S - 1)
    W0, W1, W2 = cheb_W[0], cheb_W[1], cheb_W[2]
    Wx64 = W0 + W1 / d + W2 * ((2.0 - d * d) / (d * d))
    Wg64 = -W1 / d + W2 * (2.0 * (d - 1.0) / (d * d))
    # z_j = (A.x_j) @ Wx + w8 @ WC + c_const,  w8 = A.g8
    WC64 = Wg64 - (gs.astype(np.float64)[:, None] / S) * (Wx64 + S * Wg64)
    c_const = gb.astype(np.float64) @ (Wx64 + S * Wg64) + cheb_b

    featT = np.zeros((F, NPAD), np.float16)
    featT[:, :N] = feature.T.astype(np.float16)
    wenc = W_enc.astype(np.float16)
    benc = b_enc.reshape(1, EMB).astype(np.float16)
    wx16 = Wx64.astype(np.float16)
    wc16 = WC64.astype(np.float16)
    wo16 = np.stack([W_out[:CONV, 0], W_out[CONV:, 0]], axis=1).astype(np.float16)
    eyef = np.eye(128, dtype=np.float16)
    vecs = np.zeros((128, 8), np.float32)
    vecs[:, 0] = (2.0 * gs - gs * gs) / 8.0
    vecs[:, 1] = gw
    vecs[:, 3] = c_const.astype(np.float32)
    vecs[0, 4] = b_out[0]
    vecs[:, 5] = 1e-30
    vecs[:, 6] = EPS

    shared = dict(featT=featT, wenc=wenc, benc=benc, wx=wx16, wc=wc16,
                  wo=wo16, eyef=eyef, vecs=vecs)

    in_maps = []
    for c in range(NCORES):
        base = c * ECORE
        hcol = np.zeros((EPAD, S), np.int16)
        hcol[:ECORE] = hn[base:base + ECORE].astype(np.int16)
        # one 512-idx gather per (block, member plane): idxs wrapped in 16
        # partitions, 32 columns each, laid out (b, j)-major.
        ids = np.transpose(hcol.reshape(NBLK, L, S), (0, 2, 1))  # [b, j, e]
        idxw = np.zeros((NBLK * S, 16, L // 16), np.int16)
        pos = np.arange(L)
        for t in range(NBLK * S):
            idxw[t, pos % 16, pos // 16] = ids.reshape(NBLK * S, L)[t]
        idx16 = np.tile(idxw.transpose(1, 0, 2).reshape(16, -1), (8, 1))
        in_maps.append(dict(shared, idx16=idx16))
    return in_maps


def _install_trace_hook():
    """Best-effort NTFF profiling under axon (test/benchmark only)."""
    import types
    ah = sys.modules.get("antenv.axon_hooks")
    if ah is None:
        ah = types.ModuleType("antenv.axon_hooks")
        ah._HOOK = None
        ah.set_axon_ntff_profile_hook = lambda h: setattr(ah, "_HOOK", h)
        ah.get_axon_ntff_profile_hook = lambda: ah._HOOK
        sys.modules["antenv.axon_hooks"] = ah
        import antenv
        antenv.axon_hooks = ah
    if ah.get_axon_ntff_profile_hook() is None:
        from trn_agent_boot.trn_boot import _ntff_profile_via_ctypes
        hook = _ntff_profile_via_ctypes("/opt/axon/libaxon_pjrt.so")
        if hook is not None:
            ah.set_axon_ntff_profile_hook(hook)
    import concourse.bass_utils as bu
    bu.upload_artifacts = lambda tmpdir: f"local:{tmpdir}"


def _run(in_maps, trace=False):
    nc = _get_program()
    if trace:
        _install_trace_hook()
    return run_bass_kernel_spmd(nc, in_maps, list(range(NCORES)), trace=trace)


def kernel(**inputs) -> np.ndarray:
    in_maps = _host_prep(inputs)
    res = _run(in_maps)
    out = np.concatenate([res.results[c]["yout"][:ECORE] for c in range(NCORES)])
    return out.reshape(E, 1).astype(np.float32)


def kernel_traced(**inputs):
    """Like kernel() but returns (output, exec_time_ns) using a profiled run."""
    in_maps = _host_prep(inputs)
    res = _run(in_maps, trace=True)
    out = np.concatenate([res.results[c]["yout"][:ECORE] for c in range(NCORES)])
    return out.reshape(E, 1).astype(np.float32), res.exec_time_ns
